# revision 16
# baseline (speedup 1.0000x reference)
"""Trainium2 Bass kernel for nn_DecoderLayer_83854941487838.

Qwen3-style decoder layer (RMSNorm -> QKV+QK-norm+RoPE -> causal attention
with full attn-matrix output -> Wo -> residual -> RMSNorm -> SwiGLU MLP ->
residual) plus importance-based top-k prune indices.

Tensor-parallel across 8 NeuronCores: attention heads (2/core) and MLP
intermediate dim (768/core) sharded; cross-core sums via chunked
ReduceScatter / AllGather collectives overlapped with compute.  The
importance row (attn row S-1) is recomputed in full fp32 so the top-k
prune indices match the reference ordering.  Self-contained.
"""

import math

import numpy as np

import concourse.bass as bass
import concourse.mybir as mybir
import concourse.tile as tile
from concourse import bacc, bass_utils
from concourse.alu_op_type import AluOpType
from concourse.bass import ts
from concourse.masks import make_identity, make_lower_triangular

# ---------------- problem constants (hardcoded) ----------------
B = 1
S, D = 2048, 2048
H, KV, HD = 16, 8, 128
FF = 6144
EPS = 1e-6
THETA = 1e6
PRUNE_K = int(0.3 * S)

NCORES = 8
P = 128
HPC = H // NCORES          # heads per core = 2
FFS = FF // NCORES         # MLP cols per core = 768
DBLK = D // P              # 16
FBLK = FFS // P            # 6

f32 = mybir.dt.float32
f32r = mybir.dt.float32r
bf16 = mybir.dt.bfloat16
i32 = mybir.dt.int32

SCALE = float(1.0 / np.float32(np.sqrt(np.float32(HD))))
EXACT_IMP = False

INV_FREQ = (1.0 / (THETA ** (np.arange(0, HD, 2, dtype=np.float64) / HD))
            ).astype(np.float32)                      # [64]
TWO_PI = 2.0 * math.pi
_C1 = float(np.float32(6.28125))
_r1 = TWO_PI - _C1
_C2 = float(np.float32(int(_r1 * 2**22) / 2**22))
_C3 = float(np.float32(TWO_PI - _C1 - _C2))
PI = float(np.float32(math.pi))
HALF_PI = float(np.float32(math.pi / 2))


def build_nc(nblk=S // P):
    s_len = nblk * P
    CBLK = 4 if nblk % 4 == 0 else nblk   # s-blocks per collective chunk
    NCH = nblk // CBLK                    # number of collective chunks
    CSZ = CBLK * P                        # rows per chunk
    strip = CSZ // NCORES                 # rows per core per chunk
    rows_pc = s_len // NCORES
    CW = min(512, s_len)                  # score-chunk width
    bpc = CW // P
    max_ch = max(1, s_len // CW)

    nc = bacc.Bacc("TRN2", target_bir_lowering=False, debug=False,
                   num_devices=NCORES)

    # ---------------- I/O ----------------
    hidden = nc.dram_tensor("hidden", [s_len, D], f32r, kind="ExternalInput")
    hid_rows = nc.dram_tensor("hid_rows", [rows_pc, D], f32r,
                              kind="ExternalInput")
    positions = nc.dram_tensor("positions", [s_len], i32, kind="ExternalInput")
    in_ln_w = nc.dram_tensor("in_ln_w", [D], f32r, kind="ExternalInput")
    post_ln_w = nc.dram_tensor("post_ln_w", [D], f32r, kind="ExternalInput")
    q_norm_w = nc.dram_tensor("q_norm_w", [HD], f32r, kind="ExternalInput")
    k_norm_w = nc.dram_tensor("k_norm_w", [HD], f32r, kind="ExternalInput")
    wqkv = nc.dram_tensor("wqkv", [D, 4 * HD], f32r, kind="ExternalInput")
    wo = nc.dram_tensor("wo", [2 * HD, D], f32r, kind="ExternalInput")
    wg = nc.dram_tensor("wg", [D, FFS], f32r, kind="ExternalInput")
    wu = nc.dram_tensor("wu", [D, FFS], f32r, kind="ExternalInput")
    wd = nc.dram_tensor("wd", [FFS, D], f32r, kind="ExternalInput")
    # fp32 copies for the exact importance-row path
    wq_f = nc.dram_tensor("wq_f", [D, 2 * HD], f32, kind="ExternalInput")
    wk_f = nc.dram_tensor("wk_f", [D, HD], f32, kind="ExternalInput")

    attn_part = nc.dram_tensor("attn_part", [HPC, s_len, s_len], f32r,
                               kind="ExternalOutput")
    out_part = nc.dram_tensor("out_part", [rows_pc, D], f32r,
                              kind="ExternalOutput")
    imp_rows = nc.dram_tensor("imp_rows", [HPC, s_len], f32,
                              kind="ExternalOutput")

    with tile.TileContext(nc) as tc:
        _body(nc, tc, nblk, s_len, rows_pc, CW, bpc, max_ch,
              CBLK, NCH, CSZ, strip,
              hidden, hid_rows, positions, in_ln_w, post_ln_w, q_norm_w,
              k_norm_w, wqkv, wo, wg, wu, wd, wq_f, wk_f,
              attn_part, out_part, imp_rows)
    nc.compile()
    return nc


def _body(nc, tc, nblk, s_len, rows_pc, CW, bpc, max_ch,
          CBLK, NCH, CSZ, strip,
          hidden, hid_rows, positions, in_ln_w, post_ln_w, q_norm_w,
          k_norm_w, wqkv, wo, wg, wu, wd, wq_f, wk_f,
          attn_part, out_part, imp_rows):
    X = mybir.AxisListType.X
    rg = [list(range(NCORES))]
    Af = mybir.ActivationFunctionType

    # ============ constants ============
    const_cm = tc.tile_pool(name="const", bufs=1)
    const = const_cm.__enter__()
    ident_f = const.tile([P, P], f32, name="ident_f")
    make_identity(nc, ident_f)
    ident = const.tile([P, P], f32r, name="ident")
    nc.vector.tensor_copy(ident, ident_f)
    tril = const.tile([P, P], f32, name="tril")
    make_lower_triangular(nc, tril, val=1.0, diag=True)
    eps_t = const.tile([P, 1], f32, name="eps_t")
    nc.vector.memset(eps_t, EPS)
    qw_bc = const.tile([P, HD], f32r, name="qw_bc")
    nc.sync.dma_start(qw_bc, q_norm_w.ap()[None, :].to_broadcast((P, HD)))
    kw_bc = const.tile([P, HD], f32r, name="kw_bc")
    nc.sync.dma_start(kw_bc, k_norm_w.ap()[None, :].to_broadcast((P, HD)))
    qw_bc_f = const.tile([P, HD], f32, name="qw_bc_f")
    nc.sync.dma_start(qw_bc_f, q_norm_w.ap().bitcast(f32)[None, :].to_broadcast((P, HD)))
    kw_bc_f = const.tile([P, HD], f32, name="kw_bc_f")
    nc.sync.dma_start(kw_bc_f, k_norm_w.ap().bitcast(f32)[None, :].to_broadcast((P, HD)))
    # in_ln_w reshaped [p, ko] for folding into the QKV weights
    inw_k = const.tile([P, DBLK], f32, name="inw_k")
    nc.sync.dma_start(inw_k, in_ln_w.ap().bitcast(f32).rearrange("(ko p) -> p ko", p=P))

    # post_ln broadcast (through stage E)
    lnp_cm = tc.tile_pool(name="lnp", bufs=1)
    lnp = lnp_cm.__enter__()
    postw_bc = lnp.tile([P, D], f32r, name="postw_bc")
    nc.sync.dma_start(postw_bc, post_ln_w.ap()[None, :].to_broadcast((P, D)))

    # ============ DRAM bounce buffers ============
    dram_cm = tc.tile_pool(name="dram", bufs=1, space="DRAM")
    dram = dram_cm.__enter__()
    ao_bounce = dram.tile([s_len, D], f32r, name="ao_bounce")
    hsum_b = dram.tile([NCH, strip, D], f32r, name="hsum_b")
    y_in_b = dram.tile([NCH, strip, D], f32r, name="y_in_b")
    y_b = dram.tile([s_len, D], f32r, name="y_b")
    mlp_b = dram.tile([s_len, D], f32r, name="mlp_b")
    msum_b = dram.tile([NCH, strip, D], f32r, name="msum_b")
    h_rows_b = dram.tile([NCH, strip, D], f32r, name="h_rows_b")

    # global transpose-psum pool
    psT_cm = tc.tile_pool(name="psT", bufs=2, space="PSUM")
    psT = psT_cm.__enter__()

    # ============ persistent attention operands ============
    persist_cm = tc.tile_pool(name="persist", bufs=1)
    persist = persist_cm.__enter__()
    qT = [persist.tile([P, s_len], f32r, name=f"qT{h}") for h in range(HPC)]
    kT = persist.tile([P, s_len], f32r, name="kT")
    v_bf = persist.tile([P, nblk, HD], bf16, name="v_bf")
    kT_f = persist.tile([P, s_len], f32, name="kT_f")      # exact path
    qfT = persist.tile([P, HPC, P], f32, name="qfT")       # exact path

    # ============ RoPE tables (fp32) ============
    ropep_cm = tc.tile_pool(name="ropep", bufs=1)
    ropep = ropep_cm.__enter__()
    cos_sb = ropep.tile([P, nblk, 64], f32, name="cos_sb")
    sin_sb = ropep.tile([P, nblk, 64], f32, name="sin_sb")
    with tc.tile_pool(name="ropetmp", bufs=2) as rp:
        pos_i = rp.tile([P, nblk], i32, name="pos_i")
        nc.sync.dma_start(pos_i, positions.ap().rearrange("(n p) -> p n", p=P))
        pos_f = rp.tile([P, nblk], f32, name="pos_f")
        nc.vector.tensor_copy(pos_f, pos_i)
        inv_bc = rp.tile([P, 64], f32, name="inv_bc")
        for j in range(64):
            nc.vector.memset(inv_bc[:, j:j + 1], float(INV_FREQ[j]))
        for n in range(nblk):
            ang = rp.tile([P, 64], f32, name="ang", tag="ang")
            nc.vector.tensor_tensor(
                ang, inv_bc, pos_f[:, n:n + 1].to_broadcast((P, 64)),
                op=AluOpType.mult)
            kf = rp.tile([P, 64], f32, name="kf", tag="kf")
            ki = rp.tile([P, 64], i32, name="ki", tag="ki")
            nc.vector.tensor_scalar(kf, ang, 1.0 / TWO_PI, 0.5,
                                    op0=AluOpType.mult, op1=AluOpType.add)
            nc.vector.tensor_copy(ki, kf)
            nc.vector.tensor_copy(kf, ki)
            y = rp.tile([P, 64], f32, name="y", tag="y")
            nc.vector.scalar_tensor_tensor(y, kf, -_C1, ang,
                                           op0=AluOpType.mult,
                                           op1=AluOpType.add)
            nc.vector.scalar_tensor_tensor(y, kf, -_C2, y,
                                           op0=AluOpType.mult,
                                           op1=AluOpType.add)
            nc.vector.scalar_tensor_tensor(y, kf, -_C3, y,
                                           op0=AluOpType.mult,
                                           op1=AluOpType.add)
            m = rp.tile([P, 64], f32, name="m", tag="m")
            nc.vector.tensor_scalar(m, y, PI, None, op0=AluOpType.is_gt)
            nc.vector.scalar_tensor_tensor(y, m, -TWO_PI, y,
                                           op0=AluOpType.mult,
                                           op1=AluOpType.add)
            nc.vector.tensor_scalar(m, y, -PI, None, op0=AluOpType.is_lt)
            nc.vector.scalar_tensor_tensor(y, m, TWO_PI, y,
                                           op0=AluOpType.mult,
                                           op1=AluOpType.add)
            nc.scalar.activation(sin_sb[:, n], y, Af.Sin)
            yc = rp.tile([P, 64], f32, name="yc", tag="yc")
            nc.vector.tensor_scalar(yc, y, HALF_PI, None, op0=AluOpType.add)
            nc.vector.tensor_scalar(m, yc, PI, None, op0=AluOpType.is_gt)
            nc.vector.scalar_tensor_tensor(yc, m, -TWO_PI, yc,
                                           op0=AluOpType.mult,
                                           op1=AluOpType.add)
            nc.scalar.activation(cos_sb[:, n], yc, Af.Sin)

    def rms_scale(pool, seg, width, tag):
        """Return [pr,1] fp32 reciprocal RMS of seg (pr x width)."""
        pr = seg.shape[0]
        sq_out = pool.tile([pr, width], f32r, tag=f"{tag}_sq",
                           name=f"{tag}_sq")
        ssq = pool.tile([pr, 1], f32, tag=f"{tag}_ssq", name=f"{tag}_ssq")
        nc.scalar.activation(sq_out, seg, Af.Square, accum_out=ssq)
        std = pool.tile([pr, 1], f32, tag=f"{tag}_std", name=f"{tag}_std")
        nc.scalar.activation(std, ssq, Af.Sqrt, bias=eps_t[:pr],
                             scale=1.0 / width)
        rinv = pool.tile([pr, 1], f32, tag=f"{tag}_ri", name=f"{tag}_ri")
        nc.vector.reciprocal(rinv, std)
        return rinv

    def rope(pool, qn, cos_i, sin_i, out_dtype, tag):
        ro = pool.tile([P, HD], out_dtype, tag=f"{tag}_ro", name=f"{tag}_ro")
        t1 = pool.tile([P, 64], out_dtype, tag=f"{tag}_t1", name=f"{tag}_t1")
        nc.vector.tensor_tensor(t1, qn[:, 64:], sin_i, op=AluOpType.mult)
        nc.vector.tensor_tensor(ro[:, :64], qn[:, :64], cos_i,
                                op=AluOpType.mult)
        nc.vector.tensor_sub(ro[:, :64], ro[:, :64], t1)
        nc.vector.tensor_tensor(t1, qn[:, :64], sin_i, op=AluOpType.mult)
        nc.vector.tensor_tensor(ro[:, 64:], qn[:, 64:], cos_i,
                                op=AluOpType.mult)
        nc.vector.tensor_add(ro[:, 64:], ro[:, 64:], t1)
        return ro

    # ============ stage B: QKV projection (ln weight folded) ============
    wqkv_pool_cm = tc.tile_pool(name="wqkv_pool", bufs=1)
    wqkv_pool = wqkv_pool_cm.__enter__()
    wqkv_sb = wqkv_pool.tile([P, DBLK, 4 * HD], f32r, name="wqkv_sb")
    nc.sync.dma_start(wqkv_sb, wqkv.ap().rearrange("(ko p) n -> p ko n", p=P))
    wqf_sb = wqkv_pool.tile([P, DBLK, 2 * HD], f32, name="wqf_sb")
    nc.sync.dma_start(wqf_sb, wq_f.ap().rearrange("(ko p) n -> p ko n", p=P))
    wkf_sb = wqkv_pool.tile([P, DBLK, HD], f32, name="wkf_sb")
    nc.sync.dma_start(wkf_sb, wk_f.ap().rearrange("(ko p) n -> p ko n", p=P))
    # fold in_ln_w into every weight row (exact: (x*w) @ W == x @ (w.T*W))
    for d in range(DBLK):
        nc.vector.tensor_tensor(
            wqkv_sb[:, d], wqkv_sb[:, d],
            inw_k[:, d:d + 1].to_broadcast((P, 4 * HD)), op=AluOpType.mult)
        nc.vector.tensor_tensor(
            wqf_sb[:, d], wqf_sb[:, d],
            inw_k[:, d:d + 1].to_broadcast((P, 2 * HD)), op=AluOpType.mult)
        nc.vector.tensor_tensor(
            wkf_sb[:, d], wkf_sb[:, d],
            inw_k[:, d:d + 1].to_broadcast((P, HD)), op=AluOpType.mult)

    sbB_cm = tc.tile_pool(name="sbB", bufs=2)
    sbB = sbB_cm.__enter__()
    psQ_cm = tc.tile_pool(name="psQ", bufs=2, space="PSUM")
    psQ = psQ_cm.__enter__()

    for i in range(nblk):
        hid_i = sbB.tile([P, D], f32r, tag="hid", name="hid_i")
        nc.sync.dma_start(hid_i, hidden.ap()[ts(i, P)])
        rinv = rms_scale(sbB, hid_i, D, "B")
        ps_qkv = psQ.tile([P, 4 * HD], f32, tag="qkv", name="ps_qkv")
        ps_kf = None
        if EXACT_IMP:
            ps_kf = psQ.tile([P, HD], f32, tag="kf", name="ps_kf", bufs=1)
        last = (i == nblk - 1) and EXACT_IMP
        if last:
            ps_qf = psQ.tile([P, 2 * HD], f32, tag="qf", name="ps_qf", bufs=1)
        for d in range(DBLK):
            pt = psT.tile([P, P], f32r, tag="T", name="ptB")
            nc.tensor.transpose(pt, hid_i[:, ts(d, P)], ident)
            hT_d = sbB.tile([P, P], f32r, tag="hT", name="hT_d", bufs=3)
            nc.any.tensor_copy(hT_d, pt)
            nc.tensor.matmul(ps_qkv, hT_d, wqkv_sb[:, d],
                             start=(d == 0), stop=(d == DBLK - 1))
            if EXACT_IMP:
                hTf_d = sbB.tile([P, P], f32, tag="hTf", name="hTf_d",
                                 bufs=3)
                nc.any.tensor_copy(hTf_d, pt)
                nc.tensor.matmul(ps_kf, hTf_d, wkf_sb[:, d],
                                 start=(d == 0), stop=(d == DBLK - 1))
            if last:
                nc.tensor.matmul(ps_qf, hTf_d, wqf_sb[:, d],
                                 start=(d == 0), stop=(d == DBLK - 1))
        qkv_i = sbB.tile([P, 4 * HD], f32r, tag="qkvi", name="qkv_i")
        nc.scalar.mul(qkv_i, ps_qkv, rinv)      # apply 1/rms epilogue
        if EXACT_IMP:
            kf_i = sbB.tile([P, HD], f32, tag="kfi", name="kf_i")
            nc.scalar.mul(kf_i, ps_kf, rinv)
        nc.vector.tensor_copy(v_bf[:, i], qkv_i[:, 3 * HD:4 * HD])
        cos_i = cos_sb[:, i]
        sin_i = sin_sb[:, i]
        for hh in range(3):          # q0, q1, k (fp32r path)
            seg = qkv_i[:, hh * HD:(hh + 1) * HD]
            w_bc = qw_bc if hh < 2 else kw_bc
            ri2 = rms_scale(sbB, seg, HD, "B2")
            qn = sbB.tile([P, HD], f32r, tag="qn", name="qn")
            nc.vector.scalar_tensor_tensor(qn, seg, ri2, w_bc,
                                           op0=AluOpType.mult,
                                           op1=AluOpType.mult)
            ro = rope(sbB, qn, cos_i, sin_i, f32r, "B")
            pt = psT.tile([P, P], f32r, tag="T", name="ptR")
            nc.tensor.transpose(pt, ro, ident)
            dst = qT[hh] if hh < 2 else kT
            nc.any.tensor_copy(dst[:, ts(i, P)], pt)
        # exact fp32 K path
        if not EXACT_IMP:
            continue
        ri3 = rms_scale(sbB, kf_i, HD, "B3")
        knf = sbB.tile([P, HD], f32, tag="knf", name="knf")
        nc.vector.scalar_tensor_tensor(knf, kf_i, ri3, kw_bc_f,
                                       op0=AluOpType.mult,
                                       op1=AluOpType.mult)
        rof = rope(sbB, knf, cos_i, sin_i, f32, "Bf")
        ptf = psT.tile([P, P], f32, tag="Tf", name="ptKf", bufs=1)
        nc.tensor.transpose(ptf, rof, ident_f)
        nc.any.tensor_copy(kT_f[:, ts(i, P)], ptf)
        if last:
            qf_i = sbB.tile([P, 2 * HD], f32, tag="qfi", name="qf_i")
            nc.scalar.mul(qf_i, ps_qf, rinv)
            for hh in range(HPC):
                seg = qf_i[:, hh * HD:(hh + 1) * HD]
                ri4 = rms_scale(sbB, seg, HD, "B4")
                qnf = sbB.tile([P, HD], f32, tag="qnf", name="qnf")
                nc.vector.scalar_tensor_tensor(qnf, seg, ri4, qw_bc_f,
                                               op0=AluOpType.mult,
                                               op1=AluOpType.mult)
                rqf = rope(sbB, qnf, cos_i, sin_i, f32, "Bq")
                ptf2 = psT.tile([P, P], f32, tag="Tf", name="ptQf", bufs=1)
                nc.tensor.transpose(ptf2, rqf, ident_f)
                nc.any.tensor_copy(qfT[:, hh], ptf2)

    psQ_cm.__exit__(None, None, None)
    sbB_cm.__exit__(None, None, None)
    wqkv_pool_cm.__exit__(None, None, None)
    ropep_cm.__exit__(None, None, None)

    # ============ exact importance row (fp32 scores for q = S-1) =========
    if not EXACT_IMP:
        zrow = None
    impp_cm = tc.tile_pool(name="impp", bufs=2)
    impp = impp_cm.__enter__()
    psI_cm = tc.tile_pool(name="psI", bufs=1, space="PSUM")
    psI = psI_cm.__enter__()
    for hh in range(HPC if EXACT_IMP else 0):
        # full 128-row fp32 score block; only partition row P-1 (pos S-1)
        # is consumed.  M=1 stationary loads hang walrus, so use M=128.
        erow = impp.tile([P, s_len], f32, tag="erow", name="erow", bufs=1)
        dparts_i = impp.tile([P, max_ch], f32, tag="dpi", name="dparts_i")
        for j4 in range(max_ch):
            ps_row = psI.tile([P, CW], f32, tag="row", name="ps_row")
            nc.tensor.matmul(ps_row, qfT[:, hh], kT_f[:, ts(j4, CW)],
                             start=True, stop=True)
            nc.scalar.activation(erow[:, ts(j4, CW)], ps_row, Af.Exp,
                                 scale=SCALE,
                                 accum_out=dparts_i[:, j4:j4 + 1])
        dsum = impp.tile([P, 1], f32, tag="dsum", name="dsum")
        nc.vector.reduce_sum(dsum, dparts_i, axis=X)
        rrec = impp.tile([P, 1], f32, tag="rrec", name="rrec")
        nc.vector.reciprocal(rrec, dsum)
        nrow = impp.tile([P, s_len], f32, tag="nrow", name="nrow", bufs=1)
        nc.scalar.mul(nrow, erow, rrec)
        nc.sync.dma_start(imp_rows.ap()[hh][None, :], nrow[P - 1:P])
    psI_cm.__exit__(None, None, None)
    impp_cm.__exit__(None, None, None)

    # ============ stage C/D: attention + Wo partial, chunked RS/AG ========
    wo_pool_cm = tc.tile_pool(name="wo_pool", bufs=1)
    wo_pool = wo_pool_cm.__enter__()
    wo_sb = wo_pool.tile([P, 2, D], f32r, name="wo_sb")
    nc.sync.dma_start(wo_sb, wo.ap().rearrange("(ko p) n -> p ko n", p=P))

    sbC_cm = tc.tile_pool(name="sbC", bufs=2)
    sbC = sbC_cm.__enter__()
    ep_cm = tc.tile_pool(name="ep", bufs=10)
    ep = ep_cm.__enter__()
    psS_cm = tc.tile_pool(name="psS", bufs=2, space="PSUM")
    psS = psS_cm.__enter__()
    psC_cm = tc.tile_pool(name="psC", bufs=1, space="PSUM")
    psC = psC_cm.__enter__()
    psA_cm = tc.tile_pool(name="psA", bufs=2, space="PSUM")
    psA = psA_cm.__enter__()
    sbE_cm = tc.tile_pool(name="sbE", bufs=2)
    sbE = sbE_cm.__enter__()

    def stage_e_chunk(ck):
        nc.gpsimd.collective_compute(
            "ReduceScatter", AluOpType.add, replica_groups=rg,
            ins=[ao_bounce[ts(ck, CSZ)].opt()], outs=[hsum_b[ck].opt()])
        for r0 in range(0, strip, P):
            pr = min(P, strip - r0)
            hp = sbE.tile([pr, D], f32r, tag="hp", name="hp")
            nc.sync.dma_start(hp, hsum_b[ck, r0:r0 + pr])
            hh_ = sbE.tile([pr, D], f32r, tag="hh", name="hh_")
            nc.sync.dma_start(
                hh_, hid_rows.ap()[ck * strip + r0:ck * strip + r0 + pr])
            nc.vector.tensor_add(hp, hp, hh_)
            nc.sync.dma_start(h_rows_b[ck, r0:r0 + pr], hp)
            rinvE = rms_scale(sbE, hp, D, "E")
            y_r = sbE.tile([pr, D], f32r, tag="yr", name="y_r")
            nc.vector.scalar_tensor_tensor(y_r, hp, rinvE, postw_bc[:pr],
                                           op0=AluOpType.mult,
                                           op1=AluOpType.mult)
            nc.sync.dma_start(y_in_b[ck, r0:r0 + pr], y_r)
        nc.gpsimd.collective_compute(
            "AllGather", AluOpType.bypass, replica_groups=rg,
            ins=[y_in_b[ck].opt()], outs=[y_b[ts(ck, CSZ)].opt()])

    for i in range(nblk):
        ncha = i // bpc + 1
        b_diag = i % bpc
        # --- softmax rows for both heads (normalized in SBUF) ---
        e_norm = [[None] * ncha for _ in range(HPC)]
        for h in range(HPC):
            dparts = sbC.tile([P, max_ch], f32, tag="dparts", name="dparts")
            e_chunks = []
            for j4 in range(ncha):
                ps = psS.tile([P, CW], f32, tag="sc", name="ps_sc")
                nc.tensor.matmul(ps, qT[h][:, ts(i, P)], kT[:, ts(j4, CW)],
                                 start=True, stop=True)
                e_c = ep.tile([P, CW], f32r, tag="e", name="e_c")
                if j4 < i // bpc:
                    nc.scalar.activation(e_c, ps, Af.Exp, scale=SCALE,
                                         accum_out=dparts[:, j4:j4 + 1])
                else:
                    nc.scalar.activation(e_c, ps, Af.Exp, scale=SCALE)
                    nc.vector.tensor_tensor(e_c[:, ts(b_diag, P)],
                                            e_c[:, ts(b_diag, P)], tril,
                                            op=AluOpType.mult)
                    if b_diag + 1 < bpc:
                        nc.vector.memset(
                            e_c[:, (b_diag + 1) * P:].bitcast(f32), 0.0)
                    nc.vector.reduce_sum(dparts[:, j4:j4 + 1], e_c, axis=X)
                e_chunks.append(e_c)
            denom = sbC.tile([P, 1], f32, tag="den", name="denom")
            nc.vector.reduce_sum(denom, dparts[:, :ncha], axis=X)
            recip = sbC.tile([P, 1], f32, tag="rec", name="recip")
            nc.vector.reciprocal(recip, denom)
            for j4 in range(ncha):
                e_n = ep.tile([P, CW], f32r, tag="en", name="e_n", bufs=8)
                nc.vector.tensor_scalar_mul(e_n, e_chunks[j4], recip)
                nc.sync.dma_start(attn_part.ap()[h, ts(i, P), ts(j4, CW)],
                                  e_n)
                e_norm[h][j4] = e_n
        # --- ctxT via v-stationary matmuls on normalized probs ---
        ps_cT = psC.tile([P, HPC * P], f32, tag="ctxT", name="ps_cT")
        for kb in range(i + 1):
            j4, b = divmod(kb, bpc)
            eT_pair = sbC.tile([P, HPC * P], bf16, tag="eTp", name="eT_pair",
                               bufs=4)
            for h in range(HPC):
                pt = psT.tile([P, P], f32r, tag="T", name="ptE")
                nc.tensor.transpose(pt, e_norm[h][j4][:, ts(b, P)], ident)
                nc.vector.tensor_copy(eT_pair[:, ts(h, P)], pt)
            nc.tensor.matmul(ps_cT, v_bf[:, kb], eT_pair,
                             start=(kb == 0), stop=(kb == i))
        cT_sb = sbC.tile([P, HPC * P], f32r, tag="cT", name="cT_sb")
        nc.any.tensor_copy(cT_sb, ps_cT)
        # --- Wo partial: attn_out rows i = sum_h ctx_h @ Wo_h ---
        for nch in range(D // 512):
            ps = psA.tile([P, 512], f32, tag="ao", name="ps_ao")
            for t in range(HPC):
                nc.tensor.matmul(ps, cT_sb[:, ts(t, P)], wo_sb[:, t, ts(nch, 512)],
                                 start=(t == 0), stop=(t == HPC - 1))
            ao = sbC.tile([P, 512], f32r, tag="ao_sb", name="ao_sb", bufs=3)
            nc.any.tensor_copy(ao, ps)
            nc.sync.dma_start(ao_bounce[ts(i, P), ts(nch, 512)], ao)
        if (i + 1) % CBLK == 0:
            stage_e_chunk((i + 1) // CBLK - 1)

    sbE_cm.__exit__(None, None, None)
    psA_cm.__exit__(None, None, None)
    psC_cm.__exit__(None, None, None)
    psS_cm.__exit__(None, None, None)
    ep_cm.__exit__(None, None, None)
    sbC_cm.__exit__(None, None, None)
    wo_pool_cm.__exit__(None, None, None)
    persist_cm.__exit__(None, None, None)
    lnp_cm.__exit__(None, None, None)

    # ============ stage F: SwiGLU MLP (weights-stationary), chunked =======
    wmlp_cm = tc.tile_pool(name="wmlp", bufs=1)
    wmlp = wmlp_cm.__enter__()
    wg_sb = wmlp.tile([P, DBLK, FFS], f32r, name="wg_sb")
    nc.sync.dma_start(wg_sb, wg.ap().rearrange("(ko p) n -> p ko n", p=P))
    wu_sb = wmlp.tile([P, DBLK, FFS], f32r, name="wu_sb")
    nc.sync.dma_start(wu_sb, wu.ap().rearrange("(ko p) n -> p ko n", p=P))
    wd_sb = wmlp.tile([P, FBLK, D], f32r, name="wd_sb")
    nc.sync.dma_start(wd_sb, wd.ap().rearrange("(ko p) n -> p ko n", p=P))

    sbF_cm = tc.tile_pool(name="sbF", bufs=2)
    sbF = sbF_cm.__enter__()
    sbG_cm = tc.tile_pool(name="sbG", bufs=2)
    sbG = sbG_cm.__enter__()
    psG_cm = tc.tile_pool(name="psG", bufs=1, space="PSUM")
    psG = psG_cm.__enter__()
    psD_cm = tc.tile_pool(name="psD", bufs=2, space="PSUM")
    psD = psD_cm.__enter__()

    GW = min(256, CSZ)          # moving width for gate/up (s columns)
    gpb = CSZ // GW

    for ck in range(NCH):
        for g in range(gpb):
            # transpose y rows for this half-group -> yT [d, s(GW)]
            yT = sbF.tile([P, DBLK, GW], f32r, tag="yT", name="yT", bufs=1)
            for bb in range(GW // P):
                blk = ck * CBLK + g * (GW // P) + bb
                y_i = sbF.tile([P, D], f32r, tag="yi", name="y_i", bufs=1)
                nc.sync.dma_start(y_i, y_b[ts(blk, P)])
                for d in range(DBLK):
                    pt = psT.tile([P, P], f32r, tag="T", name="ptF")
                    nc.tensor.transpose(pt, y_i[:, ts(d, P)], ident)
                    nc.any.tensor_copy(yT[:, d, ts(bb, P)], pt)
            # gate/up with stationary weights: out [ff 128, s GW]
            act = sbF.tile([P, FBLK, GW], f32r, tag="act", name="act", bufs=1)
            for fc in range(FBLK):
                ps_g = psG.tile([P, GW], f32, tag="g", name="ps_g")
                ps_u = psG.tile([P, GW], f32, tag="u", name="ps_u")
                for d in range(DBLK):
                    st = (d == 0)
                    sp = (d == DBLK - 1)
                    nc.tensor.matmul(ps_g, wg_sb[:, d, ts(fc, P)],
                                     yT[:, d], start=st, stop=sp)
                    nc.tensor.matmul(ps_u, wu_sb[:, d, ts(fc, P)],
                                     yT[:, d], start=st, stop=sp)
                sg = sbF.tile([P, GW], f32r, tag="sg", name="sg", bufs=2)
                nc.scalar.activation(sg, ps_g, Af.Silu)
                nc.vector.tensor_tensor(act[:, fc], sg, ps_u,
                                        op=AluOpType.mult)
            # down proj: lhsT = act chunks, rhs = wd
            for bb in range(GW // P):
                blk = ck * CBLK + g * (GW // P) + bb
                for nch in range(D // 512):
                    ps = psD.tile([P, 512], f32, tag="pd", name="ps_d")
                    for fc in range(FBLK):
                        nc.tensor.matmul(ps, act[:, fc, ts(bb, P)],
                                         wd_sb[:, fc, ts(nch, 512)],
                                         start=(fc == 0),
                                         stop=(fc == FBLK - 1))
                    mo = sbF.tile([P, 512], f32r, tag="mo", name="mo",
                                  bufs=2)
                    nc.any.tensor_copy(mo, ps)
                    nc.sync.dma_start(
                        mlp_b[ts(blk, P), ts(nch, 512)], mo)
        # chunked RS + final residual rows
        nc.gpsimd.collective_compute(
            "ReduceScatter", AluOpType.add, replica_groups=rg,
            ins=[mlp_b[ts(ck, CSZ)].opt()], outs=[msum_b[ck].opt()])
        for r0 in range(0, strip, P):
            pr = min(P, strip - r0)
            for dh in range(2):
                dcol = ts(dh, D // 2)
                mp = sbG.tile([pr, D // 2], f32r, tag="mp", name="mp",
                              bufs=1)
                nc.sync.dma_start(mp, msum_b[ck, r0:r0 + pr, dcol])
                hr = sbG.tile([pr, D // 2], f32r, tag="hr", name="hr",
                              bufs=1)
                nc.sync.dma_start(hr, h_rows_b[ck, r0:r0 + pr, dcol])
                nc.vector.tensor_add(mp, mp, hr)
                nc.sync.dma_start(
                    out_part.ap()[ck * strip + r0:ck * strip + r0 + pr,
                                  dcol], mp)

    psD_cm.__exit__(None, None, None)
    psG_cm.__exit__(None, None, None)
    sbG_cm.__exit__(None, None, None)
    sbF_cm.__exit__(None, None, None)
    wmlp_cm.__exit__(None, None, None)
    psT_cm.__exit__(None, None, None)
    dram_cm.__exit__(None, None, None)
    const_cm.__exit__(None, None, None)


# ---------------- host-side driver ----------------
_NC_CACHE = {}


def _get_nc(nblk):
    if nblk not in _NC_CACHE:
        _NC_CACHE[nblk] = build_nc(nblk)
    return _NC_CACHE[nblk]


def shard_inputs(inputs, nblk=S // P):
    s_len = nblk * P
    CBLK = 4 if nblk % 4 == 0 else nblk
    NCH = nblk // CBLK
    CSZ = CBLK * P
    strip = CSZ // NCORES
    hidden = np.ascontiguousarray(
        np.asarray(inputs["hidden_states"], np.float32).reshape(s_len, D))
    pos = np.ascontiguousarray(np.asarray(inputs["positions"], np.int32))
    Wq = np.asarray(inputs["Wq"], np.float32)
    Wk = np.asarray(inputs["Wk"], np.float32)
    Wv = np.asarray(inputs["Wv"], np.float32)
    Wo = np.asarray(inputs["Wo"], np.float32)
    Wg = np.asarray(inputs["W_gate"], np.float32)
    Wu = np.asarray(inputs["W_up"], np.float32)
    Wd = np.asarray(inputs["W_down"], np.float32)
    in_maps = []
    for c in range(NCORES):
        wq_c = Wq[:, c * 2 * HD:(c + 1) * 2 * HD]
        wk_c = Wk[:, c * HD:(c + 1) * HD]
        wqkv_c = np.ascontiguousarray(np.concatenate([
            wq_c, wk_c, Wv[:, c * HD:(c + 1) * HD]], axis=1))
        row_idx = np.concatenate([
            np.arange(k * CSZ + c * strip, k * CSZ + (c + 1) * strip)
            for k in range(NCH)])
        in_maps.append({
            "hidden": hidden,
            "hid_rows": np.ascontiguousarray(hidden[row_idx]),
            "positions": pos,
            "in_ln_w": np.asarray(inputs["in_ln_w"], np.float32),
            "post_ln_w": np.asarray(inputs["post_ln_w"], np.float32),
            "q_norm_w": np.asarray(inputs["q_norm_w"], np.float32),
            "k_norm_w": np.asarray(inputs["k_norm_w"], np.float32),
            "wqkv": wqkv_c,
            "wo": np.ascontiguousarray(Wo[c * 2 * HD:(c + 1) * 2 * HD]),
            "wg": np.ascontiguousarray(Wg[:, c * FFS:(c + 1) * FFS]),
            "wu": np.ascontiguousarray(Wu[:, c * FFS:(c + 1) * FFS]),
            "wd": np.ascontiguousarray(Wd[c * FFS:(c + 1) * FFS]),
            "wq_f": np.ascontiguousarray(wq_c),
            "wk_f": np.ascontiguousarray(wk_c),
        })
    return in_maps


def run(inputs, nblk=S // P, trace=False):
    s_len = nblk * P
    CBLK = 4 if nblk % 4 == 0 else nblk
    NCH = nblk // CBLK
    CSZ = CBLK * P
    strip = CSZ // NCORES
    nc = _get_nc(nblk)
    in_maps = shard_inputs(inputs, nblk)
    res = bass_utils.run_bass_kernel_spmd(
        nc, in_maps, core_ids=list(range(NCORES)), trace=trace)
    attn = np.empty((1, H, s_len, s_len), np.float32)
    imp = np.empty((H, s_len), np.float32)
    for c in range(NCORES):
        attn[0, c * HPC:(c + 1) * HPC] = res.results[c]["attn_part"]
        imp[c * HPC:(c + 1) * HPC] = res.results[c]["imp_rows"]
    # overwrite the last attention row with the exact fp32 recomputation
    if EXACT_IMP:
        attn[0, :, s_len - 1, :] = imp
    else:
        imp = attn[0, :, s_len - 1, :]
    out = np.empty((s_len, D), np.float32)
    for c in range(NCORES):
        part = res.results[c]["out_part"]
        for k in range(NCH):
            out[k * CSZ + c * strip: k * CSZ + (c + 1) * strip] = \
                part[k * strip:(k + 1) * strip]
    out = out.reshape(1, s_len, D)
    importance = (imp.sum(axis=0) / np.float32(H)).astype(np.float32)
    importance[s_len - 1] = np.inf
    k = int(0.3 * s_len)
    prune_idxs = np.argsort(importance, kind="stable")[:k].astype(np.int32)
    return (out, attn, prune_idxs), res


def kernel(**inputs):
    (out, attn, prune_idxs), _ = run(inputs)
    return out, attn, prune_idxs
